# revision 30
# baseline (speedup 1.0000x reference)
"""Trainium2 Bass/Tile kernel for a pre-norm causal decoder block.

Math (matches the jax reference):
    h   = LN1(x) * g1 + beta1
    q,k,v = per-head projections of h (D_HEAD=21, 6 heads)
    sT  = (k @ q^T) / sqrt(21)                       (scores, transposed)
    e   = exp(sT) * tril01                           (multiplicative causal mask)
    o   = (e^T @ [v | 1]) -> softmax denominator in the appended column
    x1  = x + (o / denom) @ Wo + bo
    out = x1 + relu(LN2(x1) @ (g2*W1) + b1eff) @ W2 + b2

Sharding: pure data parallelism, batch 512 -> 64 per core across 8 cores.

Layout strategy (per core) - the residual stream lives TRANSPOSED as
xT[d, (b, t)] so model-dim contractions never need PE transposes:
  - x is loaded with a casting transposing DMA (f32 -> bf16, "b t d -> d b t")
  - LN statistics are ones-vector matmuls over the partition (d) axis;
    mean/var/rstd are computed once for all 64 batches on [1, 8192] rows
    (rstd = exp(-0.5*ln(var+eps)) keeps the Act engine on a single
    activation-table set: natural_log_exp covers Ln/Exp/Relu)
  - scores use K-stacked stationaries (4 heads x 32 rows = 128) with a
    block-diagonal zero-padded moving operand, so one matmul emits four
    heads' scores; heads 4-5 go in a second K=64 matmul
  - the causal mask is a multiplicative 0/1 tril applied post-exp on DVE
    (no mask matmuls, no -inf arithmetic)
  - attention output o[t,(h,e+1)] carries the softmax denominator in an
    appended ones column of v; Wo and the FF matmuls are group-batched
    (moving N=512) and both residual adds ride as identity-matmul
    preloads of the PSUM accumulators
All matmul operands bf16, PSUM accumulation fp32.
"""

import os
import numpy as np
import ml_dtypes

from contextlib import ExitStack

import concourse.bass as bass
import concourse.bacc as bacc
import concourse.tile as tile
from concourse import mybir
from concourse.bass_utils import run_bass_kernel_spmd

BF = mybir.dt.bfloat16
F32 = mybir.dt.float32
F8 = mybir.dt.float8e4
NPBF = ml_dtypes.bfloat16

B, T, D = 512, 128, 128
NH, DH = 6, 21
DC = NH * DH  # 126
DFF = 512
NCORES = 8
BPC = B // NCORES  # 64 batches per core
G = 4              # batches per group
NG = BPC // G      # 16 groups
EPS = 1e-5
SM_SCALE = 1.0 / np.sqrt(np.float32(DH))

AF = mybir.ActivationFunctionType
ALU = mybir.AluOpType


def _bf(a):
    return np.ascontiguousarray(np.asarray(a, dtype=np.float32)).astype(NPBF)


def _prep_weights(Wq, Wk, Wv, Wo, bo, W1, b1, W2, b2, g1, beta1, g2, beta2):
    """Host-side folding/packing. Returns dict of named arrays + flags."""
    Wq = np.asarray(Wq, np.float64)
    Wk = np.asarray(Wk, np.float64)
    Wv = np.asarray(Wv, np.float64)
    g1 = np.asarray(g1, np.float64)
    g2 = np.asarray(g2, np.float64)
    beta1 = np.asarray(beta1, np.float64)
    beta2 = np.asarray(beta2, np.float64)
    W1 = np.asarray(W1, np.float64)

    Wq_f = g1[None, :, None] * Wq * SM_SCALE   # [h, d, e]
    Wk_f = g1[None, :, None] * Wk
    Wv_f = g1[None, :, None] * Wv

    # K-stacked projection weights: 4 heads (rows 32h..32h+20) and 2 heads
    wq4 = np.zeros((D, 128), np.float64)
    wk4 = np.zeros((D, 128), np.float64)
    wq2 = np.zeros((D, 64), np.float64)
    wk2 = np.zeros((D, 64), np.float64)
    for h in range(4):
        wq4[:, 32 * h:32 * h + DH] = Wq_f[h]
        wk4[:, 32 * h:32 * h + DH] = Wk_f[h]
    for h in range(2):
        wq2[:, 32 * h:32 * h + DH] = Wq_f[4 + h]
        wk2[:, 32 * h:32 * h + DH] = Wk_f[4 + h]
    wv = np.concatenate([Wv_f[h] for h in range(NH)], axis=1)  # [128, 126]

    # beta1 contributions (per-stack-row biases for q/k; per-(h,e) row for v)
    qb = np.einsum("d,hde->he", beta1, Wq) * SM_SCALE   # [6, 21]
    kb = np.einsum("d,hde->he", beta1, Wk)
    vb = np.einsum("d,hde->he", beta1, Wv)
    qb4 = np.zeros((128, 1), np.float64)
    kb4 = np.zeros((128, 1), np.float64)
    qb2 = np.zeros((64, 1), np.float64)
    kb2 = np.zeros((64, 1), np.float64)
    for h in range(4):
        qb4[32 * h:32 * h + DH, 0] = qb[h]
        kb4[32 * h:32 * h + DH, 0] = kb[h]
    for h in range(2):
        qb2[32 * h:32 * h + DH, 0] = qb[4 + h]
        kb2[32 * h:32 * h + DH, 0] = kb[4 + h]

    w1 = g2[:, None] * W1                     # [128, 512]
    b1_eff = np.asarray(b1, np.float64) + beta2 @ W1   # [512]
    w2c = np.asarray(W2, np.float64).reshape(4, 128, D).transpose(1, 0, 2)  # [128,4,128]

    mask01 = np.where(
        np.arange(T)[:, None] <= np.arange(T)[None, :], 1.0, 0.0
    )  # [s, t] keep where s <= t

    out = {
        "wq4": _bf(wq4), "wq2": _bf(wq2), "wk4": _bf(wk4), "wk2": _bf(wk2),
        "wv": _bf(wv), "wo": _bf(Wo), "w1": _bf(w1), "w2c": _bf(w2c),
        "mask01": _bf(mask01), "ident": _bf(np.eye(128)),
        "ones_bf": _bf(np.ones((D, 1))),
        "qb4": np.asarray(qb4, np.float32), "qb2": np.asarray(qb2, np.float32),
        "kb4": np.asarray(kb4, np.float32), "kb2": np.asarray(kb2, np.float32),
        "vb_r": np.asarray(vb.reshape(1, DC), np.float32),
        "bo_c": np.asarray(bo, np.float32).reshape(D, 1),
        "b2_c": np.asarray(b2, np.float32).reshape(D, 1),
        "beta1_c": np.asarray(beta1, np.float32).reshape(D, 1),
        "b1e": np.ascontiguousarray(
            np.asarray(b1_eff, np.float64).reshape(4, 128).T, dtype=np.float32
        ),  # [128, 4] per-partition relu bias per chunk
    }
    flags = {
        "qkv_bias": bool(np.any(beta1 != 0.0)),
        "beta1": bool(np.any(beta1 != 0.0)),
        "bo": bool(np.any(np.asarray(bo) != 0.0)),
        "b2": bool(np.any(np.asarray(b2) != 0.0)),
        "b1": bool(np.any(out["b1e"] != 0.0)),
    }
    return out, flags


def _emit(ctx, tc, aps, flags, bpc):
    nc = tc.nc
    x_ap = aps["x"]
    y_ap = aps["y"]

    singles = ctx.enter_context(tc.tile_pool(name="singles", bufs=1))
    sbg = ctx.enter_context(tc.tile_pool(name="sbg", bufs=2))
    psA = ctx.enter_context(tc.tile_pool(name="psA", bufs=1, space="PSUM"))
    psS = ctx.enter_context(tc.tile_pool(name="psS", bufs=1, space="PSUM"))
    psO = ctx.enter_context(tc.tile_pool(name="psO", bufs=1, space="PSUM"))
    psM = ctx.enter_context(tc.tile_pool(name="psM", bufs=1, space="PSUM"))
    psF = ctx.enter_context(tc.tile_pool(name="psF", bufs=1, space="PSUM"))

    def load_const(name, shape, dtype=BF):
        t = singles.tile(list(shape), dtype, tag=name, name=name)
        nc.sync.dma_start(out=t[:], in_=aps[name])
        return t

    wq4 = load_const("wq4", [D, 128])
    wq2 = load_const("wq2", [D, 64])
    wk4 = load_const("wk4", [D, 128])
    wk2 = load_const("wk2", [D, 64])
    wv = load_const("wv", [D, DC])
    wo = load_const("wo", [DC, D])
    w1 = load_const("w1", [D, DFF])
    w2c = load_const("w2c", [D, 4, D])
    mask01 = load_const("mask01", [T, T])
    ident = load_const("ident", [128, 128])
    ones_bf = load_const("ones_bf", [D, 1])
    if flags["qkv_bias"]:
        qb4 = load_const("qb4", [128, 1], F32)
        qb2 = load_const("qb2", [64, 1], F32)
        kb4 = load_const("kb4", [128, 1], F32)
        kb2 = load_const("kb2", [64, 1], F32)
        vb_r = load_const("vb_r", [1, DC], F32)
    if flags["beta1"]:
        beta1_c = load_const("beta1_c", [D, 1], F32)
    if flags["bo"]:
        bo_c = load_const("bo_c", [D, 1], F32)
    if flags["b2"]:
        b2_c = load_const("b2_c", [D, 1], F32)
    if flags["b1"]:
        b1e = load_const("b1e", [128, 4], F32)

    # ---- whole-core resident tensors -------------------------------------
    stage = singles.tile([T, bpc, D], BF, tag="stage")   # load/store staging
    xb, oS = stage, stage
    xTb = singles.tile([D, bpc, T], BF, tag="xTb")       # bf16 residual in
    xx1b = singles.tile([D, bpc, T], BF, tag="xx1b")     # post-attn residual
    outT = singles.tile([D, bpc, T], BF, tag="outT")     # final output (T)
    # LN stat rows (all on partition 0; shared by LN1/LN2 - the per-group
    # broadcasts read the DRAM copies)
    st_sums = singles.tile([1, NG * 512], BF, tag="st_sums")
    st_sq = singles.tile([1, NG * 512], BF, tag="st_sq")
    st_mu = singles.tile([1, NG * 512], BF, tag="st_mu")
    st_rstd = singles.tile([1, NG * 512], BF, tag="st_rstd")

    # block-diagonal moving operands for the K-stacked score matmuls;
    # off-block zeros are written once and never touched again
    qblk4 = singles.tile([128, G, 4, T], BF, tag="qblk4")
    qblk2 = singles.tile([64, G, 2, T], BF, tag="qblk2")
    # fp8 block-diag AV moving operand: [s, b, head-pair, ktile, 2*(DH+1)]
    v8 = singles.tile([T, G, 3, 2, 2 * (DH + 1)], F8, tag="v8")
    k4sb = singles.tile([128, G, T], BF, tag="k4sb")
    k2sb = singles.tile([64, G, T], BF, tag="k2sb")

    nc.vector.memset(qblk4[:], 0.0)
    nc.vector.memset(qblk2[:], 0.0)
    nc.vector.memset(v8[:], 0.0)
    nc.gpsimd.memset(v8[:, :, :, 0, DH:DH + 1], 1.0)
    nc.gpsimd.memset(v8[:, :, :, 1, 2 * DH + 1:2 * DH + 2], 1.0)
    eps_t = singles.tile([4, 1], F32, tag="eps")
    nc.vector.memset(eps_t[:], EPS)

    NQ = 4           # DMA quarters
    BQ = bpc // NQ   # 16 batches per quarter

    # eT column offset of head h for pair-slot j (see spair bank layout)
    def ecol(j, h):
        if j == 0:
            return 128 * h if h < 4 else 512 + 128 * (h - 4)
        return 1024 + 128 * h if h < 4 else 768 + 128 * (h - 4)

    stats1_dram = nc.dram_tensor("stats1_dram", [2, NG * 512], BF, kind="Internal").ap()
    stats2_dram = nc.dram_tensor("stats2_dram", [2, NG * 512], BF, kind="Internal").ap()

    def ln_stats(src, stats_dram):
        """Per-(b,t)-column mean/rstd over the partition (d) axis."""
        xsqg = None
        for g in range(NG):
            cols = slice(512 * g, 512 * (g + 1))
            if g % 2 == 0:
                xsqg = sbg.tile([D, 2 * G, T], BF, tag="xsqg", name="xsqg")
                s2 = src[:, G * g:G * (g + 2), :]
                nc.vector.tensor_tensor(out=xsqg[:], in0=s2, in1=s2, op=ALU.mult)
            s = src[:, G * g:G * (g + 1), :]
            stp = psM.tile([1, 512], F32, tag="m", name="stp")
            nc.tensor.matmul(
                stp[:], ones_bf[:],
                s.rearrange("d g t -> d (g t)"),
                start=True, stop=True, skip_group_check=True,
            )
            stp2 = psO.tile([1, 512], F32, tag="o", name="stp2")
            nc.tensor.matmul(
                stp2[:], ones_bf[:],
                xsqg[:, G * (g % 2):G * (g % 2 + 1), :].rearrange("d g t -> d (g t)"),
                start=True, stop=True, skip_group_check=True,
            )
            nc.vector.tensor_copy(out=st_sums[0:1, cols], in_=stp[:])
            nc.vector.tensor_copy(out=st_sq[0:1, cols], in_=stp2[:])
        nc.vector.tensor_scalar_mul(out=st_mu[:], in0=st_sums[:], scalar1=1.0 / D)
        nc.vector.tensor_scalar_mul(out=st_sq[:], in0=st_sq[:], scalar1=1.0 / D)
        nc.vector.tensor_mul(out=st_sums[:], in0=st_mu[:], in1=st_mu[:])
        nc.vector.tensor_sub(out=st_sq[:], in0=st_sq[:], in1=st_sums[:])
        nc.scalar.activation(out=st_sq[:], in_=st_sq[:], func=AF.Ln, bias=eps_t[0:1, :])
        nc.scalar.activation(out=st_rstd[:], in_=st_sq[:], func=AF.Exp, scale=-0.5)
        nc.sync.dma_start(out=stats_dram[0:1, :], in_=st_mu[:])
        nc.sync.dma_start(out=stats_dram[1:2, :], in_=st_rstd[:])

    def normalize(src, stats_dram, gp, out_t, add_beta1):
        """out_t[d, (2g t)] = (src - mu) * rstd  (+ beta1), for group pair gp."""
        cols = slice(1024 * gp, 1024 * (gp + 1))
        # DVE cannot broadcast across partitions: replicate the stat rows
        # (via DRAM, whose APs allow a zero partition step)
        rep = sbg.tile([D, 2, 1024], BF, tag="rep", name="rep")
        nc.sync.dma_start(
            out=rep[:, 0, :], in_=stats_dram[0:1, cols].to_broadcast([D, 1024])
        )
        nc.sync.dma_start(
            out=rep[:, 1, :], in_=stats_dram[1:2, cols].to_broadcast([D, 1024])
        )
        s = src[:, 2 * G * gp:2 * G * (gp + 1), :].rearrange("d g t -> d (g t)")
        o = out_t[:].rearrange("d g t -> d (g t)")
        nc.vector.tensor_tensor(out=o, in0=s, in1=rep[:, 0, :], op=ALU.subtract)
        nc.vector.tensor_tensor(out=o, in0=o, in1=rep[:, 1, :], op=ALU.mult)
        if add_beta1:
            nc.vector.tensor_scalar_add(out=o, in0=o, scalar1=beta1_c[:])

    def emit_once():
        # ---- P0: load + LN1 stats ----------------------------------------
        nc.gpsimd.dma_start(out=xb[:], in_=x_ap.rearrange("b t d -> t b d"))
        for b in range(bpc):
            nc.sync.dma_start_transpose(out=xTb[:, b, :], in_=xb[:, b, :])
        ln_stats(xTb, stats1_dram)

        # ---- P1: attention per group -------------------------------------
        hh2 = None
        for g in range(NG):
            if g % 2 == 0:
                hh2 = sbg.tile([D, 2 * G, T], BF, tag="hhT", name="hh2")
                normalize(xTb, stats1_dram, g // 2, hh2, flags["beta1"])
            hhT = hh2[:, G * (g % 2):G * (g % 2 + 1), :]
            hhflat = hhT.rearrange("d g t -> d (g t)")

            # q/k projections (K-stacked rows) + block-diag staging
            q4_ps = psA.tile([128, G, T], F32, tag="a", name="q4_ps")
            nc.tensor.matmul(
                q4_ps[:].rearrange("p g t -> p (g t)"), wq4[:], hhflat,
                start=True, stop=True,
            )
            for h in range(4):
                src = q4_ps[32 * h:32 * h + 32, :, :]
                dst = qblk4[32 * h:32 * h + 32, :, h, :]
                if flags["qkv_bias"]:
                    nc.vector.tensor_scalar_add(
                        out=dst, in0=src, scalar1=qb4[32 * h:32 * h + 32, :]
                    )
                else:
                    nc.vector.tensor_copy(out=dst, in_=src)
            q2_ps = psA.tile([64, G, T], F32, tag="a", name="q2_ps")
            nc.tensor.matmul(
                q2_ps[:].rearrange("p g t -> p (g t)"), wq2[:], hhflat,
                start=True, stop=True,
            )
            for h in range(2):
                src = q2_ps[32 * h:32 * h + 32, :, :]
                dst = qblk2[32 * h:32 * h + 32, :, h, :]
                if flags["qkv_bias"]:
                    nc.vector.tensor_scalar_add(
                        out=dst, in0=src, scalar1=qb2[32 * h:32 * h + 32, :]
                    )
                else:
                    nc.vector.tensor_copy(out=dst, in_=src)
            k4_ps = psA.tile([128, G, T], F32, tag="a", name="k4_ps")
            nc.tensor.matmul(
                k4_ps[:].rearrange("p g t -> p (g t)"), wk4[:], hhflat,
                start=True, stop=True,
            )
            if flags["qkv_bias"]:
                nc.vector.tensor_scalar_add(out=k4sb[:], in0=k4_ps[:], scalar1=kb4[:])
            else:
                nc.vector.tensor_copy(out=k4sb[:], in_=k4_ps[:])
            k2_ps = psA.tile([64, G, T], F32, tag="a", name="k2_ps")
            nc.tensor.matmul(
                k2_ps[:].rearrange("p g t -> p (g t)"), wk2[:], hhflat,
                start=True, stop=True,
            )
            if flags["qkv_bias"]:
                nc.vector.tensor_scalar_add(out=k2sb[:], in0=k2_ps[:], scalar1=kb2[:])
            else:
                nc.vector.tensor_copy(out=k2sb[:], in_=k2_ps[:])

            v_ps = psA.tile([T, G, DC], F32, tag="a", name="v_ps")
            for b in range(G):
                nc.tensor.matmul(
                    v_ps[:, b, :], hhT[:, b, :], wv[:],
                    start=True, stop=True, skip_group_check=True,
                )
            # v values land block-diagonally in the fp8 DoubleRow moving
            # operand: head h=2p+j -> k-tile j, columns 22j..22j+20
            vv = v_ps[:].rearrange("t g (p j e) -> t g p j e", p=3, j=2)
            va = v8[:, :, :, 0, 0:DH]
            v8dst = bass.AP(
                tensor=va.tensor, offset=va.offset,
                ap=[va.ap[0], va.ap[1], va.ap[2], [2 * (DH + 1) + DH + 1, 2],
                    va.ap[3]],
            )
            if flags["qkv_bias"]:
                vb3 = vb_r[:].rearrange("o (p j e) -> o p j e", p=3, j=2)
                vb4 = bass.AP(
                    tensor=vb3.tensor, offset=vb3.offset,
                    ap=[[0, T], [0, G], vb3.ap[1], vb3.ap[2], vb3.ap[3]],
                )
                nc.vector.tensor_tensor(out=v8dst, in0=vv, in1=vb4, op=ALU.add)
            else:
                nc.vector.tensor_copy(out=v8dst, in_=vv)

            # attention pairs: scores -> exp -> mask -> AV -> softmax divide
            oT_ps = psM.tile([DC, G, T], BF, tag="m", name="oT_ps")
            for p in range(2):
                sp = psS.tile([T, 1536], F32, tag="s", name="sp")
                for j in range(2):
                    b = 2 * p + j
                    off4 = 0 if j == 0 else 1024
                    off2 = 512 if j == 0 else 768
                    nc.tensor.matmul(
                        sp[:, off4:off4 + 512],
                        k4sb[:, b, :],
                        qblk4[:, b, :, :].rearrange("p h t -> p (h t)"),
                        start=True, stop=True, skip_group_check=True,
                    )
                    nc.tensor.matmul(
                        sp[:, off2:off2 + 256],
                        k2sb[:, b, :],
                        qblk2[:, b, :, :].rearrange("p h t -> p (h t)"),
                        start=True, stop=True, skip_group_check=True,
                    )
                eT = sbg.tile([T, 1536], BF, tag="eT", name="eT")
                nc.scalar.activation(out=eT[:], in_=sp[:], func=AF.Exp)
                mb = bass.AP(
                    tensor=mask01.tensor, offset=mask01.offset,
                    ap=[mask01.ap[0], [0, 12], mask01.ap[1]],
                )
                # the causal mask multiply doubles as the fp8 downcast for
                # the DoubleRow AV matmuls
                eT8 = sbg.tile([T, 1536], F8, tag="eT8", name="eT8")
                eTv = eT[:].rearrange("t (m c) -> t m c", c=T)
                e8v = eT8[:].rearrange("t (m c) -> t m c", c=T)
                nc.vector.tensor_tensor(out=e8v, in0=eTv, in1=mb, op=ALU.mult)

                # 2 heads per AV matmul: K=256 via fp8 DoubleRow k-tiles
                o_ps = psO.tile([T, 2, 3, 2 * (DH + 1)], F32, tag="o", name="o_ps")
                for j in range(2):
                    b = 2 * p + j
                    for hp in range(3):
                        c = ecol(j, 2 * hp)
                        nc.tensor.matmul(
                            o_ps[:, j, hp, :],
                            eT8[:, c:c + 2 * T].rearrange("s (k t) -> s k t", k=2),
                            v8[:, b, hp, :, :],
                            start=True, stop=True, skip_group_check=True,
                            perf_mode=mybir.MatmulPerfMode.DoubleRow,
                        )
                ov = o_ps[:].rearrange("t j p (u e) -> t j p u e", u=2)
                recip = sbg.tile([T, 2, 3, 2, 1], F32, tag="recip", name="recip")
                nc.vector.reciprocal(out=recip[:], in_=ov[:, :, :, :, DH:DH + 1])
                o_sb = sbg.tile([T, 2, NH, DH], BF, tag="o_sb", name="o_sb")
                nc.vector.tensor_tensor(
                    out=o_sb[:].rearrange("t j (p u) e -> t j p u e", u=2),
                    in0=ov[:, :, :, :, 0:DH],
                    in1=recip[:].to_broadcast([T, 2, 3, 2, DH]), op=ALU.mult,
                )
                for j in range(2):
                    b = 2 * p + j
                    nc.tensor.transpose(
                        out=oT_ps[:, b, :],
                        in_=o_sb[:, j, :, :].rearrange("t h e -> t (h e)"),
                        identity=ident[:],
                    )
            oT_sb = sbg.tile([DC, G, T], BF, tag="oT_sb", name="oT_sb")
            nc.vector.tensor_copy(out=oT_sb[:], in_=oT_ps[:])

            # x1T = xT + Wo^T @ oT  (mixed f32-psum + bf16 residual add)
            att = psM.tile([D, G * T], F32, tag="m", name="att")
            nc.tensor.matmul(
                att[:], wo[:], oT_sb[:].rearrange("c g t -> c (g t)"),
                start=True, stop=True, skip_group_check=True,
            )
            x1o = xx1b[:, G * g:G * (g + 1), :].rearrange("d g t -> d (g t)")
            xres = xTb[:, G * g:G * (g + 1), :].rearrange("d g t -> d (g t)")
            nc.vector.tensor_tensor(out=x1o, in0=att[:], in1=xres, op=ALU.add)
            if flags["bo"]:
                nc.vector.tensor_scalar_add(out=x1o, in0=x1o, scalar1=bo_c[:])

        # ---- P2: LN2 stats ----------------------------------------------
        ln_stats(xx1b, stats2_dram)

        # ---- P3: feed-forward per group ----------------------------------
        h22 = None
        for g in range(NG):
            if g % 2 == 0:
                h22 = sbg.tile([D, 2 * G, T], BF, tag="h2T", name="h22")
                normalize(xx1b, stats2_dram, g // 2, h22, False)
            h2T = h22[:, G * (g % 2):G * (g % 2 + 1), :]
            h2flat = h2T.rearrange("d g t -> d (g t)")
            r_sb = sbg.tile([128, 4, 512], BF, tag="r_sb", name="r_sb")
            for i in range(2):
                fp = psF.tile([128, 2, 512], F32, tag="f", name="fp")
                for c in range(2):
                    nc.tensor.matmul(
                        fp[:, c, :], w1[:, 128 * (2 * i + c):128 * (2 * i + c + 1)],
                        h2flat, start=True, stop=True, skip_group_check=True,
                    )
                if flags["b1"]:
                    for c in range(2):
                        nc.scalar.activation(
                            out=r_sb[:, 2 * i + c, :], in_=fp[:, c, :], func=AF.Relu,
                            bias=b1e[:, 2 * i + c:2 * i + c + 1],
                        )
                else:
                    nc.scalar.activation(
                        out=r_sb[:, 2 * i:2 * i + 2, :].rearrange("p c t -> p (c t)"),
                        in_=fp[:].rearrange("p c t -> p (c t)"), func=AF.Relu,
                    )
            fo = psM.tile([D, G * T], F32, tag="m", name="fo")
            for c in range(4):
                nc.tensor.matmul(
                    fo[:], w2c[:, c, :], r_sb[:, c, :],
                    start=(c == 0), stop=(c == 3), skip_group_check=True,
                )
            oo = outT[:, G * g:G * (g + 1), :].rearrange("d g t -> d (g t)")
            x1res = xx1b[:, G * g:G * (g + 1), :].rearrange("d g t -> d (g t)")
            nc.vector.tensor_tensor(out=oo, in0=fo[:], in1=x1res, op=ALU.add)
            if flags["b2"]:
                nc.vector.tensor_scalar_add(out=oo, in0=oo, scalar1=b2_c[:])

        # ---- P4: transpose back + store ----------------------------------
        for b in range(bpc):
            nc.sync.dma_start_transpose(out=oS[:, b, :], in_=outT[:, b, :])
        nc.gpsimd.dma_start(out=y_ap.rearrange("b t d -> t b d"), in_=oS[:])

    repeat = int(os.environ.get("K_REPEAT", "1"))
    for _ in range(repeat):
        emit_once()


def build_program(weights, flags, bpc=BPC):
    nc = bacc.Bacc("TRN2", target_bir_lowering=False, debug=False)
    aps = {}
    aps["x"] = nc.dram_tensor("x", [bpc, T, D], F32, kind="ExternalInput").ap()
    aps["y"] = nc.dram_tensor("y", [bpc, T, D], F32, kind="ExternalOutput").ap()
    for name, arr in weights.items():
        dt = F32 if arr.dtype == np.float32 else BF
        aps[name] = nc.dram_tensor(name, list(arr.shape), dt, kind="ExternalInput").ap()
    with tile.TileContext(nc) as tc:
        with ExitStack() as ctx:
            _emit(ctx, tc, aps, flags, bpc)
    nc.compile()
    return nc


_CACHE = {}


def _get_program_and_maps(x, args):
    x = np.asarray(x, np.float32)
    weights, flags = _prep_weights(*args)
    key = tuple(sorted(flags.items()))
    if key not in _CACHE:
        _CACHE[key] = build_program(weights, flags)
    nc = _CACHE[key]
    in_maps = []
    for c in range(NCORES):
        m = {"x": np.ascontiguousarray(x[c * BPC:(c + 1) * BPC])}
        m.update(weights)
        in_maps.append(m)
    return nc, in_maps


def kernel(x, Wq, Wk, Wv, Wo, bo, W1, b1, W2, b2, g1, beta1, g2, beta2):
    nc, in_maps = _get_program_and_maps(
        x, (Wq, Wk, Wv, Wo, bo, W1, b1, W2, b2, g1, beta1, g2, beta2)
    )
    res = run_bass_kernel_spmd(nc, in_maps, list(range(NCORES)))
    out = np.concatenate([res.results[c]["y"] for c in range(NCORES)], axis=0)
    return out.astype(np.float32)


def run_traced(inputs):
    """Profiled run; returns BassKernelResults with exec_time_ns if available."""
    args = tuple(
        inputs[k]
        for k in ("Wq", "Wk", "Wv", "Wo", "bo", "W1", "b1", "W2", "b2",
                  "g1", "beta1", "g2", "beta2")
    )
    nc, in_maps = _get_program_and_maps(inputs["x"], args)
    return run_bass_kernel_spmd(nc, in_maps, list(range(NCORES)), trace=True)


# revision 31
# speedup vs baseline: 1.2542x; 1.2542x over previous
"""Trainium2 Bass/Tile kernel for a pre-norm causal decoder block.

Math (matches the jax reference):
    h   = LN1(x) * g1 + beta1
    q,k,v = per-head projections of h (D_HEAD=21, 6 heads)
    sT  = (k @ q^T) / sqrt(21)                       (scores, transposed)
    e   = exp(sT) * tril01                           (multiplicative causal mask)
    o   = (e^T @ [v | 1]) -> softmax denominator in the appended column
    x1  = x + (o / denom) @ Wo + bo
    out = x1 + relu(LN2(x1) @ (g2*W1) + b1eff) @ W2 + b2

Sharding: pure data parallelism, batch 512 -> 64 per core across 8 cores.

Layout strategy (per core) - the residual stream lives TRANSPOSED as
xT[d, (b, t)] so model-dim contractions never need PE transposes:
  - x is loaded with a casting transposing DMA (f32 -> bf16, "b t d -> d b t")
  - LN statistics are ones-vector matmuls over the partition (d) axis;
    mean/var/rstd are computed once for all 64 batches on [1, 8192] rows
    (rstd = exp(-0.5*ln(var+eps)) keeps the Act engine on a single
    activation-table set: natural_log_exp covers Ln/Exp/Relu)
  - scores use K-stacked stationaries (4 heads x 32 rows = 128) with a
    block-diagonal zero-padded moving operand, so one matmul emits four
    heads' scores; heads 4-5 go in a second K=64 matmul
  - the causal mask is a multiplicative 0/1 tril applied post-exp on DVE
    (no mask matmuls, no -inf arithmetic)
  - attention output o[t,(h,e+1)] carries the softmax denominator in an
    appended ones column of v; Wo and the FF matmuls are group-batched
    (moving N=512) and both residual adds ride as identity-matmul
    preloads of the PSUM accumulators
All matmul operands bf16, PSUM accumulation fp32.
"""

import os
import numpy as np
import ml_dtypes

from contextlib import ExitStack

import concourse.bass as bass
import concourse.bacc as bacc
import concourse.tile as tile
from concourse import mybir
from concourse.bass_utils import run_bass_kernel_spmd

BF = mybir.dt.bfloat16
F32 = mybir.dt.float32
NPBF = ml_dtypes.bfloat16

B, T, D = 512, 128, 128
NH, DH = 6, 21
DC = NH * DH  # 126
DFF = 512
NCORES = 8
BPC = B // NCORES  # 64 batches per core
G = 4              # batches per group
NG = BPC // G      # 16 groups
EPS = 1e-5
SM_SCALE = 1.0 / np.sqrt(np.float32(DH))

AF = mybir.ActivationFunctionType
ALU = mybir.AluOpType


def _bf(a):
    return np.ascontiguousarray(np.asarray(a, dtype=np.float32)).astype(NPBF)


def _prep_weights(Wq, Wk, Wv, Wo, bo, W1, b1, W2, b2, g1, beta1, g2, beta2):
    """Host-side folding/packing. Returns dict of named arrays + flags."""
    Wq = np.asarray(Wq, np.float64)
    Wk = np.asarray(Wk, np.float64)
    Wv = np.asarray(Wv, np.float64)
    g1 = np.asarray(g1, np.float64)
    g2 = np.asarray(g2, np.float64)
    beta1 = np.asarray(beta1, np.float64)
    beta2 = np.asarray(beta2, np.float64)
    W1 = np.asarray(W1, np.float64)

    Wq_f = g1[None, :, None] * Wq * SM_SCALE   # [h, d, e]
    Wk_f = g1[None, :, None] * Wk
    Wv_f = g1[None, :, None] * Wv

    # K-stacked projection weights: 4 heads (rows 32h..32h+20) and 2 heads
    wq4 = np.zeros((D, 128), np.float64)
    wk4 = np.zeros((D, 128), np.float64)
    wq2 = np.zeros((D, 64), np.float64)
    wk2 = np.zeros((D, 64), np.float64)
    for h in range(4):
        wq4[:, 32 * h:32 * h + DH] = Wq_f[h]
        wk4[:, 32 * h:32 * h + DH] = Wk_f[h]
    for h in range(2):
        wq2[:, 32 * h:32 * h + DH] = Wq_f[4 + h]
        wk2[:, 32 * h:32 * h + DH] = Wk_f[4 + h]
    wv = np.concatenate([Wv_f[h] for h in range(NH)], axis=1)  # [128, 126]

    # beta1 contributions (per-stack-row biases for q/k; per-(h,e) row for v)
    qb = np.einsum("d,hde->he", beta1, Wq) * SM_SCALE   # [6, 21]
    kb = np.einsum("d,hde->he", beta1, Wk)
    vb = np.einsum("d,hde->he", beta1, Wv)
    qb4 = np.zeros((128, 1), np.float64)
    kb4 = np.zeros((128, 1), np.float64)
    qb2 = np.zeros((64, 1), np.float64)
    kb2 = np.zeros((64, 1), np.float64)
    for h in range(4):
        qb4[32 * h:32 * h + DH, 0] = qb[h]
        kb4[32 * h:32 * h + DH, 0] = kb[h]
    for h in range(2):
        qb2[32 * h:32 * h + DH, 0] = qb[4 + h]
        kb2[32 * h:32 * h + DH, 0] = kb[4 + h]

    w1 = g2[:, None] * W1                     # [128, 512]
    b1_eff = np.asarray(b1, np.float64) + beta2 @ W1   # [512]
    w2c = np.asarray(W2, np.float64).reshape(4, 128, D).transpose(1, 0, 2)  # [128,4,128]

    mask01 = np.where(
        np.arange(T)[:, None] <= np.arange(T)[None, :], 1.0, 0.0
    )  # [s, t] keep where s <= t

    out = {
        "wq4": _bf(wq4), "wq2": _bf(wq2), "wk4": _bf(wk4), "wk2": _bf(wk2),
        "wv": _bf(wv), "wo": _bf(Wo), "w1": _bf(w1), "w2c": _bf(w2c),
        "mask01": _bf(mask01), "ident": _bf(np.eye(128)),
        "ones_bf": _bf(np.ones((D, 1))),
        "qb4": np.asarray(qb4, np.float32), "qb2": np.asarray(qb2, np.float32),
        "kb4": np.asarray(kb4, np.float32), "kb2": np.asarray(kb2, np.float32),
        "vb_r": np.asarray(vb.reshape(1, DC), np.float32),
        "bo_c": np.asarray(bo, np.float32).reshape(D, 1),
        "b2_c": np.asarray(b2, np.float32).reshape(D, 1),
        "beta1_c": np.asarray(beta1, np.float32).reshape(D, 1),
        "b1e": np.ascontiguousarray(
            np.asarray(b1_eff, np.float64).reshape(4, 128).T, dtype=np.float32
        ),  # [128, 4] per-partition relu bias per chunk
    }
    flags = {
        "qkv_bias": bool(np.any(beta1 != 0.0)),
        "beta1": bool(np.any(beta1 != 0.0)),
        "bo": bool(np.any(np.asarray(bo) != 0.0)),
        "b2": bool(np.any(np.asarray(b2) != 0.0)),
        "b1": bool(np.any(out["b1e"] != 0.0)),
    }
    return out, flags


def _emit(ctx, tc, aps, flags, bpc):
    nc = tc.nc
    x_ap = aps["x"]
    y_ap = aps["y"]

    singles = ctx.enter_context(tc.tile_pool(name="singles", bufs=1))
    sbg = ctx.enter_context(tc.tile_pool(name="sbg", bufs=2))
    psA = ctx.enter_context(tc.tile_pool(name="psA", bufs=1, space="PSUM"))
    psS = ctx.enter_context(tc.tile_pool(name="psS", bufs=1, space="PSUM"))
    psO = ctx.enter_context(tc.tile_pool(name="psO", bufs=1, space="PSUM"))
    psM = ctx.enter_context(tc.tile_pool(name="psM", bufs=1, space="PSUM"))
    psF = ctx.enter_context(tc.tile_pool(name="psF", bufs=1, space="PSUM"))

    def load_const(name, shape, dtype=BF):
        t = singles.tile(list(shape), dtype, tag=name, name=name)
        nc.sync.dma_start(out=t[:], in_=aps[name])
        return t

    wq4 = load_const("wq4", [D, 128])
    wq2 = load_const("wq2", [D, 64])
    wk4 = load_const("wk4", [D, 128])
    wk2 = load_const("wk2", [D, 64])
    wv = load_const("wv", [D, DC])
    wo = load_const("wo", [DC, D])
    w1 = load_const("w1", [D, DFF])
    w2c = load_const("w2c", [D, 4, D])
    mask01 = load_const("mask01", [T, T])
    ident = load_const("ident", [128, 128])
    ones_bf = load_const("ones_bf", [D, 1])
    if flags["qkv_bias"]:
        qb4 = load_const("qb4", [128, 1], F32)
        qb2 = load_const("qb2", [64, 1], F32)
        kb4 = load_const("kb4", [128, 1], F32)
        kb2 = load_const("kb2", [64, 1], F32)
        vb_r = load_const("vb_r", [1, DC], F32)
    if flags["beta1"]:
        beta1_c = load_const("beta1_c", [D, 1], F32)
    if flags["bo"]:
        bo_c = load_const("bo_c", [D, 1], F32)
    if flags["b2"]:
        b2_c = load_const("b2_c", [D, 1], F32)
    if flags["b1"]:
        b1e = load_const("b1e", [128, 4], F32)

    # ---- whole-core resident tensors -------------------------------------
    stage = singles.tile([T, bpc, D], BF, tag="stage")   # load/store staging
    xb, oS = stage, stage
    xTb = singles.tile([D, bpc, T], BF, tag="xTb")       # bf16 residual in
    xx1b = singles.tile([D, bpc, T], BF, tag="xx1b")     # post-attn residual
    outT = singles.tile([D, bpc, T], BF, tag="outT")     # final output (T)
    # LN stat rows (all on partition 0; shared by LN1/LN2 - the per-group
    # broadcasts read the DRAM copies)
    st_sums = singles.tile([1, NG * 512], BF, tag="st_sums")
    st_sq = singles.tile([1, NG * 512], BF, tag="st_sq")
    st_mu = singles.tile([1, NG * 512], BF, tag="st_mu")
    st_rstd = singles.tile([1, NG * 512], BF, tag="st_rstd")

    # block-diagonal moving operands for the K-stacked score matmuls;
    # off-block zeros are written once and never touched again
    qblk4 = singles.tile([128, G, 4, T], BF, tag="qblk4")
    qblk2 = singles.tile([64, G, 2, T], BF, tag="qblk2")
    v_sb = singles.tile([T, G, NH, DH + 1], BF, tag="v_sb")
    k4sb = singles.tile([128, G, T], BF, tag="k4sb")
    k2sb = singles.tile([64, G, T], BF, tag="k2sb")

    nc.vector.memset(qblk4[:], 0.0)
    nc.vector.memset(qblk2[:], 0.0)
    nc.gpsimd.memset(v_sb[:, :, :, DH:DH + 1], 1.0)
    eps_t = singles.tile([4, 1], F32, tag="eps")
    nc.vector.memset(eps_t[:], EPS)

    NQ = 4           # DMA quarters
    BQ = bpc // NQ   # 16 batches per quarter

    # eT column offset of head h for pair-slot j (see spair bank layout)
    def ecol(j, h):
        if j == 0:
            return 128 * h if h < 4 else 512 + 128 * (h - 4)
        return 1024 + 128 * h if h < 4 else 768 + 128 * (h - 4)

    stats1_dram = nc.dram_tensor("stats1_dram", [2, NG * 512], BF, kind="Internal").ap()
    stats2_dram = nc.dram_tensor("stats2_dram", [2, NG * 512], BF, kind="Internal").ap()

    def ln_stats(src, stats_dram):
        """Per-(b,t)-column mean/rstd over the partition (d) axis."""
        xsqg = None
        for g in range(NG):
            cols = slice(512 * g, 512 * (g + 1))
            if g % 2 == 0:
                xsqg = sbg.tile([D, 2 * G, T], BF, tag="xsqg", name="xsqg")
                s2 = src[:, G * g:G * (g + 2), :]
                nc.vector.tensor_tensor(out=xsqg[:], in0=s2, in1=s2, op=ALU.mult)
            s = src[:, G * g:G * (g + 1), :]
            stp = psM.tile([1, 512], F32, tag="m", name="stp")
            nc.tensor.matmul(
                stp[:], ones_bf[:],
                s.rearrange("d g t -> d (g t)"),
                start=True, stop=True, skip_group_check=True,
            )
            stp2 = psO.tile([1, 512], F32, tag="o", name="stp2")
            nc.tensor.matmul(
                stp2[:], ones_bf[:],
                xsqg[:, G * (g % 2):G * (g % 2 + 1), :].rearrange("d g t -> d (g t)"),
                start=True, stop=True, skip_group_check=True,
            )
            nc.vector.tensor_copy(out=st_sums[0:1, cols], in_=stp[:])
            nc.vector.tensor_copy(out=st_sq[0:1, cols], in_=stp2[:])
        nc.vector.tensor_scalar_mul(out=st_mu[:], in0=st_sums[:], scalar1=1.0 / D)
        nc.vector.tensor_scalar_mul(out=st_sq[:], in0=st_sq[:], scalar1=1.0 / D)
        nc.vector.tensor_mul(out=st_sums[:], in0=st_mu[:], in1=st_mu[:])
        nc.vector.tensor_sub(out=st_sq[:], in0=st_sq[:], in1=st_sums[:])
        nc.scalar.activation(out=st_sq[:], in_=st_sq[:], func=AF.Ln, bias=eps_t[0:1, :])
        nc.scalar.activation(out=st_rstd[:], in_=st_sq[:], func=AF.Exp, scale=-0.5)
        nc.sync.dma_start(out=stats_dram[0:1, :], in_=st_mu[:])
        nc.sync.dma_start(out=stats_dram[1:2, :], in_=st_rstd[:])

    def normalize(src, stats_dram, gp, out_t, add_beta1):
        """out_t[d, (2g t)] = (src - mu) * rstd  (+ beta1), for group pair gp."""
        cols = slice(1024 * gp, 1024 * (gp + 1))
        # DVE cannot broadcast across partitions: replicate the stat rows
        # (via DRAM, whose APs allow a zero partition step)
        rep = sbg.tile([D, 2, 1024], BF, tag="rep", name="rep")
        nc.sync.dma_start(
            out=rep[:, 0, :], in_=stats_dram[0:1, cols].to_broadcast([D, 1024])
        )
        nc.sync.dma_start(
            out=rep[:, 1, :], in_=stats_dram[1:2, cols].to_broadcast([D, 1024])
        )
        s = src[:, 2 * G * gp:2 * G * (gp + 1), :].rearrange("d g t -> d (g t)")
        o = out_t[:].rearrange("d g t -> d (g t)")
        nc.vector.tensor_tensor(out=o, in0=s, in1=rep[:, 0, :], op=ALU.subtract)
        nc.vector.tensor_tensor(out=o, in0=o, in1=rep[:, 1, :], op=ALU.mult)
        if add_beta1:
            nc.vector.tensor_scalar_add(out=o, in0=o, scalar1=beta1_c[:])

    def emit_once():
        # ---- P0: load + LN1 stats ----------------------------------------
        nc.gpsimd.dma_start(out=xb[:], in_=x_ap.rearrange("b t d -> t b d"))
        for b in range(bpc):
            nc.sync.dma_start_transpose(out=xTb[:, b, :], in_=xb[:, b, :])
        ln_stats(xTb, stats1_dram)

        # ---- P1: attention per group -------------------------------------
        hh2 = None
        for g in range(NG):
            if g % 2 == 0:
                hh2 = sbg.tile([D, 2 * G, T], BF, tag="hhT", name="hh2")
                normalize(xTb, stats1_dram, g // 2, hh2, flags["beta1"])
            hhT = hh2[:, G * (g % 2):G * (g % 2 + 1), :]
            hhflat = hhT.rearrange("d g t -> d (g t)")

            # q/k projections (K-stacked rows) + block-diag staging
            q4_ps = psA.tile([128, G, T], F32, tag="a", name="q4_ps")
            nc.tensor.matmul(
                q4_ps[:].rearrange("p g t -> p (g t)"), wq4[:], hhflat,
                start=True, stop=True,
            )
            for h in range(4):
                src = q4_ps[32 * h:32 * h + 32, :, :]
                dst = qblk4[32 * h:32 * h + 32, :, h, :]
                if flags["qkv_bias"]:
                    nc.vector.tensor_scalar_add(
                        out=dst, in0=src, scalar1=qb4[32 * h:32 * h + 32, :]
                    )
                else:
                    nc.vector.tensor_copy(out=dst, in_=src)
            q2_ps = psA.tile([64, G, T], F32, tag="a", name="q2_ps")
            nc.tensor.matmul(
                q2_ps[:].rearrange("p g t -> p (g t)"), wq2[:], hhflat,
                start=True, stop=True,
            )
            for h in range(2):
                src = q2_ps[32 * h:32 * h + 32, :, :]
                dst = qblk2[32 * h:32 * h + 32, :, h, :]
                if flags["qkv_bias"]:
                    nc.vector.tensor_scalar_add(
                        out=dst, in0=src, scalar1=qb2[32 * h:32 * h + 32, :]
                    )
                else:
                    nc.vector.tensor_copy(out=dst, in_=src)
            k4_ps = psA.tile([128, G, T], F32, tag="a", name="k4_ps")
            nc.tensor.matmul(
                k4_ps[:].rearrange("p g t -> p (g t)"), wk4[:], hhflat,
                start=True, stop=True,
            )
            if flags["qkv_bias"]:
                nc.vector.tensor_scalar_add(out=k4sb[:], in0=k4_ps[:], scalar1=kb4[:])
            else:
                nc.vector.tensor_copy(out=k4sb[:], in_=k4_ps[:])
            k2_ps = psA.tile([64, G, T], F32, tag="a", name="k2_ps")
            nc.tensor.matmul(
                k2_ps[:].rearrange("p g t -> p (g t)"), wk2[:], hhflat,
                start=True, stop=True,
            )
            if flags["qkv_bias"]:
                nc.vector.tensor_scalar_add(out=k2sb[:], in0=k2_ps[:], scalar1=kb2[:])
            else:
                nc.vector.tensor_copy(out=k2sb[:], in_=k2_ps[:])

            v_ps = psA.tile([T, G, DC], F32, tag="a", name="v_ps")
            for b in range(G):
                nc.tensor.matmul(
                    v_ps[:, b, :], hhT[:, b, :], wv[:],
                    start=True, stop=True, skip_group_check=True,
                )
            vv = v_ps[:].rearrange("t g (h e) -> t g h e", h=NH)
            if flags["qkv_bias"]:
                vb3 = vb_r[:].rearrange("o (h e) -> o h e", h=NH)
                vb4 = bass.AP(
                    tensor=vb3.tensor, offset=vb3.offset,
                    ap=[[0, T], [0, G], vb3.ap[1], vb3.ap[2]],
                )
                nc.vector.tensor_tensor(
                    out=v_sb[:, :, :, 0:DH], in0=vv, in1=vb4, op=ALU.add
                )
            else:
                nc.vector.tensor_copy(out=v_sb[:, :, :, 0:DH], in_=vv)

            # attention pairs: scores -> exp -> mask -> AV -> softmax divide
            oT_ps = psM.tile([DC, G, T], BF, tag="m", name="oT_ps")
            for p in range(2):
                sp = psS.tile([T, 1536], F32, tag="s", name="sp")
                for j in range(2):
                    b = 2 * p + j
                    off4 = 0 if j == 0 else 1024
                    off2 = 512 if j == 0 else 768
                    nc.tensor.matmul(
                        sp[:, off4:off4 + 512],
                        k4sb[:, b, :],
                        qblk4[:, b, :, :].rearrange("p h t -> p (h t)"),
                        start=True, stop=True, skip_group_check=True,
                    )
                    nc.tensor.matmul(
                        sp[:, off2:off2 + 256],
                        k2sb[:, b, :],
                        qblk2[:, b, :, :].rearrange("p h t -> p (h t)"),
                        start=True, stop=True, skip_group_check=True,
                    )
                eT = sbg.tile([T, 1536], BF, tag="eT", name="eT")
                nc.scalar.activation(out=eT[:], in_=sp[:], func=AF.Exp)
                mb = bass.AP(
                    tensor=mask01.tensor, offset=mask01.offset,
                    ap=[mask01.ap[0], [0, 12], mask01.ap[1]],
                )
                eTv = eT[:].rearrange("t (m c) -> t m c", c=T)
                nc.vector.tensor_tensor(out=eTv, in0=eTv, in1=mb, op=ALU.mult)

                o_ps = psO.tile([T, 2, NH, DH + 1], F32, tag="o", name="o_ps")
                for j in range(2):
                    b = 2 * p + j
                    for h in range(NH):
                        c = ecol(j, h)
                        nc.tensor.matmul(
                            o_ps[:, j, h, :], eT[:, c:c + T], v_sb[:, b, h, :],
                            start=True, stop=True, skip_group_check=True,
                        )
                recip = sbg.tile([T, 2, NH, 1], F32, tag="recip", name="recip")
                nc.vector.reciprocal(out=recip[:], in_=o_ps[:, :, :, DH:DH + 1])
                o_sb = sbg.tile([T, 2, NH, DH], BF, tag="o_sb", name="o_sb")
                nc.vector.tensor_tensor(
                    out=o_sb[:], in0=o_ps[:, :, :, 0:DH],
                    in1=recip[:].to_broadcast([T, 2, NH, DH]), op=ALU.mult,
                )
                for j in range(2):
                    b = 2 * p + j
                    nc.tensor.transpose(
                        out=oT_ps[:, b, :],
                        in_=o_sb[:, j, :, :].rearrange("t h e -> t (h e)"),
                        identity=ident[:],
                    )
            oT_sb = sbg.tile([DC, G, T], BF, tag="oT_sb", name="oT_sb")
            nc.vector.tensor_copy(out=oT_sb[:], in_=oT_ps[:])

            # x1T = xT + Wo^T @ oT  (mixed f32-psum + bf16 residual add)
            att = psM.tile([D, G * T], F32, tag="m", name="att")
            nc.tensor.matmul(
                att[:], wo[:], oT_sb[:].rearrange("c g t -> c (g t)"),
                start=True, stop=True, skip_group_check=True,
            )
            x1o = xx1b[:, G * g:G * (g + 1), :].rearrange("d g t -> d (g t)")
            xres = xTb[:, G * g:G * (g + 1), :].rearrange("d g t -> d (g t)")
            nc.vector.tensor_tensor(out=x1o, in0=att[:], in1=xres, op=ALU.add)
            if flags["bo"]:
                nc.vector.tensor_scalar_add(out=x1o, in0=x1o, scalar1=bo_c[:])

        # ---- P2: LN2 stats ----------------------------------------------
        ln_stats(xx1b, stats2_dram)

        # ---- P3: feed-forward per group ----------------------------------
        h22 = None
        for g in range(NG):
            if g % 2 == 0:
                h22 = sbg.tile([D, 2 * G, T], BF, tag="h2T", name="h22")
                normalize(xx1b, stats2_dram, g // 2, h22, False)
            h2T = h22[:, G * (g % 2):G * (g % 2 + 1), :]
            h2flat = h2T.rearrange("d g t -> d (g t)")
            r_sb = sbg.tile([128, 4, 512], BF, tag="r_sb", name="r_sb")
            for i in range(2):
                fp = psF.tile([128, 2, 512], F32, tag="f", name="fp")
                for c in range(2):
                    nc.tensor.matmul(
                        fp[:, c, :], w1[:, 128 * (2 * i + c):128 * (2 * i + c + 1)],
                        h2flat, start=True, stop=True, skip_group_check=True,
                    )
                if flags["b1"]:
                    for c in range(2):
                        nc.scalar.activation(
                            out=r_sb[:, 2 * i + c, :], in_=fp[:, c, :], func=AF.Relu,
                            bias=b1e[:, 2 * i + c:2 * i + c + 1],
                        )
                else:
                    nc.scalar.activation(
                        out=r_sb[:, 2 * i:2 * i + 2, :].rearrange("p c t -> p (c t)"),
                        in_=fp[:].rearrange("p c t -> p (c t)"), func=AF.Relu,
                    )
            fo = psM.tile([D, G * T], F32, tag="m", name="fo")
            for c in range(4):
                nc.tensor.matmul(
                    fo[:], w2c[:, c, :], r_sb[:, c, :],
                    start=(c == 0), stop=(c == 3), skip_group_check=True,
                )
            oo = outT[:, G * g:G * (g + 1), :].rearrange("d g t -> d (g t)")
            x1res = xx1b[:, G * g:G * (g + 1), :].rearrange("d g t -> d (g t)")
            nc.vector.tensor_tensor(out=oo, in0=fo[:], in1=x1res, op=ALU.add)
            if flags["b2"]:
                nc.vector.tensor_scalar_add(out=oo, in0=oo, scalar1=b2_c[:])

        # ---- P4: transpose back + store ----------------------------------
        for b in range(bpc):
            nc.sync.dma_start_transpose(out=oS[:, b, :], in_=outT[:, b, :])
        nc.gpsimd.dma_start(out=y_ap.rearrange("b t d -> t b d"), in_=oS[:])

    repeat = int(os.environ.get("K_REPEAT", "1"))
    for _ in range(repeat):
        emit_once()


def build_program(weights, flags, bpc=BPC):
    nc = bacc.Bacc("TRN2", target_bir_lowering=False, debug=False)
    aps = {}
    aps["x"] = nc.dram_tensor("x", [bpc, T, D], F32, kind="ExternalInput").ap()
    aps["y"] = nc.dram_tensor("y", [bpc, T, D], F32, kind="ExternalOutput").ap()
    for name, arr in weights.items():
        dt = F32 if arr.dtype == np.float32 else BF
        aps[name] = nc.dram_tensor(name, list(arr.shape), dt, kind="ExternalInput").ap()
    with tile.TileContext(nc) as tc:
        with ExitStack() as ctx:
            _emit(ctx, tc, aps, flags, bpc)
    nc.compile()
    return nc


_CACHE = {}


def _get_program_and_maps(x, args):
    x = np.asarray(x, np.float32)
    weights, flags = _prep_weights(*args)
    key = tuple(sorted(flags.items()))
    if key not in _CACHE:
        _CACHE[key] = build_program(weights, flags)
    nc = _CACHE[key]
    in_maps = []
    for c in range(NCORES):
        m = {"x": np.ascontiguousarray(x[c * BPC:(c + 1) * BPC])}
        m.update(weights)
        in_maps.append(m)
    return nc, in_maps


def kernel(x, Wq, Wk, Wv, Wo, bo, W1, b1, W2, b2, g1, beta1, g2, beta2):
    nc, in_maps = _get_program_and_maps(
        x, (Wq, Wk, Wv, Wo, bo, W1, b1, W2, b2, g1, beta1, g2, beta2)
    )
    res = run_bass_kernel_spmd(nc, in_maps, list(range(NCORES)))
    out = np.concatenate([res.results[c]["y"] for c in range(NCORES)], axis=0)
    return out.astype(np.float32)


def run_traced(inputs):
    """Profiled run; returns BassKernelResults with exec_time_ns if available."""
    args = tuple(
        inputs[k]
        for k in ("Wq", "Wk", "Wv", "Wo", "bo", "W1", "b1", "W2", "b2",
                  "g1", "beta1", "g2", "beta2")
    )
    nc, in_maps = _get_program_and_maps(inputs["x"], args)
    return run_bass_kernel_spmd(nc, in_maps, list(range(NCORES)), trace=True)
